# revision 4
# baseline (speedup 1.0000x reference)
"""Multi-head attention kernel for Trainium2 (8 NeuronCores, Bass/Tile).

Problem: B=2, S=2048, D=1024, H=16 heads (HD=64), causal mask, fp32.
Reference quirk: V is projected from the ALREADY-projected keys:
    k = keys @ Wk + bk ; v = k @ Wv + bv  =>  v = keys @ (Wk@Wv) + (bk@Wv + bv)

Sharding: core c handles batch b = c//4 and head-group g = c%4 (4 heads,
head-feature columns [256g, 256g+256)).  Each core:
  - projects q/k/v for its heads from its batch (contraction over full D),
  - computes full-sequence attention for its 4 heads,
  - produces a partial output  attn_g @ Wo[rows of g]  (row-parallel Wo).
Host sums the 4 partials per batch and adds bo.

Device layouts (per core):
  xqT/xkT   [D, S]  fp32   (host-transposed activations)
  qT/kT     [128, 2*S] bf16  (head-feat on partitions; hf-block hb -> cols hb*S+tok)
  v_aug     [128, NKC*512] bf16 (per k-chunk of 128 toks: per head [v_h(64)|ones(64)])
  S^T tile  psum [128 ktok, 512 qtok] = kT_h.T @ qT_h   (contraction over HD=64)
  P^T tile  bf16 [128, 512] = exp(S^T/8) with causal zeroing/masking
  PV        psum [128, 512] = v_aug_h.T @ P^T  accum over k-chunks:
              rows 0:64 = unnormalized attn^T, rows 64:128 = replicated row-sums
  attnT     [128, 2*S] bf16 = normalized attn^T  (DVE: rows0:64 * recip(rows64:128))
  out chunk psum [128 tok, 512 of] = attnT.T @ Wo  accum over 2 hf-blocks
"""
import sys
sys.path.insert(0, "/opt/trn_rl_repo")

import numpy as np
import ml_dtypes

import concourse.bacc as bacc
import concourse.mybir as mybir
import concourse.tile as tile
from concourse.bass_utils import run_bass_kernel_spmd

F32 = mybir.dt.float32
F32R = mybir.dt.float32r
BF16 = mybir.dt.bfloat16
AF = mybir.ActivationFunctionType

B, S, D, H, HD = 2, 2048, 1024, 16, 64
NCORES = 8
HPC = 4            # heads per core
HF = HPC * HD      # 256 head-features per core
NKC = S // 128     # 16 k-chunks of 128 tokens
NQB = S // 512     # 4 q-blocks of 512 tokens
NDC = D // 128     # 8 contraction chunks for projections
SCALE = 1.0 / np.sqrt(HD)


def _classify_mask(mask):
    """Per (qblock 512, kchunk 128) x (qsub 128) classification of mask^T.

    Returns (plan, mask_tiles):
      plan[qb][kc] = None (fully masked -> skip) or (subs, c0, c1) where
        subs[j] in {('Z',), ('F',), ('M', idx)} and [c0, c1) is the exp span.
      mask_tiles: list of distinct [128,128] 0/1 int tiles (transposed: [kt, qt]).
    """
    maskT = np.ascontiguousarray(mask.T)
    tiles = {}
    tiles_list = []
    plan = []
    for qb in range(NQB):
        row = []
        for kc in range(NKC):
            subT = maskT[kc * 128:(kc + 1) * 128, qb * 512:(qb + 1) * 512]
            subs = []
            for j in range(4):
                blk = subT[:, j * 128:(j + 1) * 128]
                if not blk.any():
                    subs.append(('Z',))
                elif blk.all():
                    subs.append(('F',))
                else:
                    key = blk.tobytes()
                    if key not in tiles:
                        tiles[key] = len(tiles_list)
                        tiles_list.append(blk)
                    subs.append(('M', tiles[key]))
            if all(s[0] == 'Z' for s in subs):
                row.append(None)
            else:
                nz = [j for j, s in enumerate(subs) if s[0] != 'Z']
                row.append((subs, nz[0] * 128, (nz[-1] + 1) * 128))
        plan.append(row)
    return plan, tiles_list


def _build_nc(plan, nmt, has_vbias):
    nc = bacc.Bacc("TRN2", target_bir_lowering=False, debug=False)

    xqT = nc.dram_tensor("xqT", [D, S], F32R, kind="ExternalInput").ap()
    xkT = nc.dram_tensor("xkT", [D, S], F32R, kind="ExternalInput").ap()
    wq_d = nc.dram_tensor("wq", [D, HF], F32R, kind="ExternalInput").ap()
    wk_d = nc.dram_tensor("wk", [D, HF], F32R, kind="ExternalInput").ap()
    wkv_d = nc.dram_tensor("wkv", [D, HF], F32R, kind="ExternalInput").ap()
    wo_d = nc.dram_tensor("wo", [HF, D], BF16, kind="ExternalInput").ap()
    bq_d = nc.dram_tensor("bq", [128, 2], F32, kind="ExternalInput").ap()
    bk_d = nc.dram_tensor("bk", [128, 2], F32, kind="ExternalInput").ap()
    bkv_d = nc.dram_tensor("bkv", [1, HF], F32R, kind="ExternalInput").ap()
    mt_d = nc.dram_tensor("mtiles", [max(nmt, 1), 128, 128], BF16,
                          kind="ExternalInput").ap()
    out_d = nc.dram_tensor("out", [S, D], F32, kind="ExternalOutput").ap()

    with tile.TileContext(nc) as tc:
        with tc.tile_pool(name="wpool", bufs=1) as wpool, \
             tc.tile_pool(name="big", bufs=1) as big, \
             tc.tile_pool(name="xpool", bufs=6) as xpool, \
             tc.tile_pool(name="ptpool", bufs=4) as ptpool, \
             tc.tile_pool(name="npool", bufs=2) as npool, \
             tc.tile_pool(name="opool", bufs=3) as opool, \
             tc.tile_pool(name="pspool", bufs=8, space="PSUM") as pspool:

            # ---------------- weights / constants ----------------
            wq_sb = wpool.tile([128, NDC * HF], F32R, tag="wq")
            wk_sb = wpool.tile([128, NDC * HF], F32R, tag="wk")
            wkv_sb = wpool.tile([128, NDC * HF], F32R, tag="wkv")
            for kc in range(NDC):
                nc.sync.dma_start(wq_sb[:, kc * HF:(kc + 1) * HF],
                                  wq_d[kc * 128:(kc + 1) * 128, :])
                nc.sync.dma_start(wk_sb[:, kc * HF:(kc + 1) * HF],
                                  wk_d[kc * 128:(kc + 1) * 128, :])
                nc.sync.dma_start(wkv_sb[:, kc * HF:(kc + 1) * HF],
                                  wkv_d[kc * 128:(kc + 1) * 128, :])
            wo_sb = wpool.tile([128, 2 * D], BF16, tag="wo")
            for hb in range(2):
                nc.sync.dma_start(wo_sb[:, hb * D:(hb + 1) * D],
                                  wo_d[hb * 128:(hb + 1) * 128, :])
            bq_sb = wpool.tile([128, 2], F32, tag="bq")
            bk_sb = wpool.tile([128, 2], F32, tag="bk")
            nc.sync.dma_start(bq_sb[:], bq_d)
            nc.sync.dma_start(bk_sb[:], bk_d)
            if nmt > 0:
                mt_sb = wpool.tile([128, nmt * 128], BF16, tag="mt")
                for i in range(nmt):
                    nc.sync.dma_start(mt_sb[:, i * 128:(i + 1) * 128], mt_d[i])
            if has_vbias:
                ones_sb = wpool.tile([1, 128], F32R, tag="ones")
                bkv_sb = wpool.tile([1, HF], F32R, tag="bkv")
                nc.gpsimd.memset(ones_sb[:], 1.0)
                nc.sync.dma_start(bkv_sb[:], bkv_d)

            # ---------------- persistent activations ----------------
            qT_sb = big.tile([128, 2 * S], BF16, tag="qT")
            kT_sb = big.tile([128, 2 * S], BF16, tag="kT")
            vaug_sb = big.tile([128, NKC * 512], BF16, tag="vaug")
            attnT_sb = big.tile([128, 2 * S], BF16, tag="attnT")

            # ones blocks of v_aug: per kchunk, per head: cols [.. +64 .. +128)
            for kc in range(NKC):
                for h in range(HPC):
                    nc.gpsimd.memset(
                        vaug_sb[:, kc * 512 + h * 128 + 64: kc * 512 + h * 128 + 128],
                        1.0)

            # ---------------- projections ----------------
            # q projection: qT[hb, tok] accumulated over 8 D-chunks
            for tci in range(S // 512):
                psq = [pspool.tile([128, 512], F32, tag="ps", name=f"psq{tci}_{i}") for i in range(2)]
                for kc in range(NDC):
                    xq_t = xpool.tile([128, 512], F32R, tag="x")
                    nc.sync.dma_start(
                        xq_t[:], xqT[kc * 128:(kc + 1) * 128,
                                     tci * 512:(tci + 1) * 512])
                    for hb in range(2):
                        nc.tensor.matmul(
                            psq[hb][:],
                            wq_sb[:, kc * HF + hb * 128: kc * HF + (hb + 1) * 128]
                            ,
                            xq_t[:],
                            start=(kc == 0), stop=(kc == NDC - 1))
                for hb in range(2):
                    nc.scalar.activation(
                        qT_sb[:, hb * S + tci * 512: hb * S + (tci + 1) * 512],
                        psq[hb][:], AF.Identity,
                        bias=bq_sb[:, hb:hb + 1], scale=1.0)

            # k and v projections share the streamed xkT tiles
            for tci in range(S // 512):
                psk = [pspool.tile([128, 512], F32, tag="ps", name=f"psk{tci}_{i}") for i in range(2)]
                psv = [pspool.tile([128, HF], F32, tag="ps", name=f"psv{tci}_{i}") for i in range(4)]
                for kc in range(NDC):
                    xk_t = xpool.tile([128, 512], F32R, tag="x")
                    nc.sync.dma_start(
                        xk_t[:], xkT[kc * 128:(kc + 1) * 128,
                                     tci * 512:(tci + 1) * 512])
                    for hb in range(2):
                        nc.tensor.matmul(
                            psk[hb][:],
                            wk_sb[:, kc * HF + hb * 128: kc * HF + (hb + 1) * 128]
                            ,
                            xk_t[:],
                            start=(kc == 0), stop=(kc == NDC - 1))
                    for ts in range(4):
                        nc.tensor.matmul(
                            psv[ts][:],
                            xk_t[:, ts * 128:(ts + 1) * 128],
                            wkv_sb[:, kc * HF:(kc + 1) * HF],
                            start=(kc == 0),
                            stop=(kc == NDC - 1 and not has_vbias))
                for hb in range(2):
                    nc.scalar.activation(
                        kT_sb[:, hb * S + tci * 512: hb * S + (tci + 1) * 512],
                        psk[hb][:], AF.Identity,
                        bias=bk_sb[:, hb:hb + 1], scale=1.0)
                for ts in range(4):
                    if has_vbias:
                        nc.tensor.matmul(psv[ts][:], ones_sb[:],
                                         bkv_sb[:],
                                         start=False, stop=True)
                    kci = tci * 4 + ts
                    for h in range(HPC):
                        nc.vector.tensor_copy(
                            vaug_sb[:, kci * 512 + h * 128: kci * 512 + h * 128 + 64],
                            psv[ts][:, h * 64:(h + 1) * 64])

            # ---------------- attention + output projection ----------------
            for qb in range(NQB):
                q0 = qb * 512
                for h in range(HPC):
                    hb, hr = h // 2, (h % 2) * 64
                    kcs = [kc for kc in range(NKC) if plan[qb][kc] is not None]
                    pv_ps = pspool.tile([128, 512], F32, tag="ps")
                    for kc in kcs:
                        subs, c0, c1 = plan[qb][kc]
                        st_ps = pspool.tile([128, 512], F32, tag="ps")
                        nc.tensor.matmul(
                            st_ps[:],
                            kT_sb[hr:hr + 64,
                                  hb * S + kc * 128: hb * S + (kc + 1) * 128],
                            qT_sb[hr:hr + 64, hb * S + q0: hb * S + q0 + 512],
                            start=True, stop=True)
                        pt = ptpool.tile([128, 512], BF16, tag="pt")
                        nc.scalar.activation(pt[:, c0:c1], st_ps[:, c0:c1],
                                             AF.Exp, bias=0.0, scale=float(SCALE))
                        if c0 > 0:
                            nc.gpsimd.memset(pt[:, 0:c0], 0.0)
                        if c1 < 512:
                            nc.gpsimd.memset(pt[:, c1:512], 0.0)
                        for j, sub in enumerate(subs):
                            lo, hi = j * 128, (j + 1) * 128
                            if sub[0] == 'Z' and lo >= c0 and hi <= c1:
                                nc.gpsimd.memset(pt[:, lo:hi], 0.0)
                            elif sub[0] == 'M':
                                idx = sub[1]
                                nc.vector.tensor_mul(
                                    pt[:, lo:hi], pt[:, lo:hi],
                                    mt_sb[:, idx * 128:(idx + 1) * 128])
                        nc.tensor.matmul(
                            pv_ps[:],
                            vaug_sb[:, kc * 512 + h * 128: kc * 512 + (h + 1) * 128],
                            pt[:],
                            start=(kc == kcs[0]), stop=(kc == kcs[-1]))
                    recip = npool.tile([64, 512], F32, tag="recip")
                    nc.vector.reciprocal(recip[:], pv_ps[64:128, :])
                    nc.vector.tensor_mul(
                        attnT_sb[hr:hr + 64, hb * S + q0: hb * S + q0 + 512],
                        pv_ps[0:64, :], recip[:])

                # output projection for this q-block
                for t in range(4):
                    tok0 = q0 + t * 128
                    for of in range(2):
                        ops = pspool.tile([128, 512], F32, tag="ps")
                        for hb2 in range(2):
                            nc.tensor.matmul(
                                ops[:],
                                attnT_sb[:, hb2 * S + tok0: hb2 * S + tok0 + 128],
                                wo_sb[:, hb2 * D + of * 512: hb2 * D + (of + 1) * 512],
                                start=(hb2 == 0), stop=(hb2 == 1))
                        obuf = opool.tile([128, 512], F32, tag="obuf")
                        nc.vector.tensor_copy(obuf[:], ops[:])
                        nc.sync.dma_start(
                            out_d[tok0:tok0 + 128, of * 512:(of + 1) * 512],
                            obuf[:])
    nc.compile()
    return nc


_CACHE = {}


def _get_nc(plan, nmt, has_vbias):
    key = (repr(plan), nmt, has_vbias)
    if key not in _CACHE:
        _CACHE[key] = _build_nc(plan, nmt, has_vbias)
    return _CACHE[key]


def shard_inputs(queries, keys, mask, Wq, bq, Wk, bk, Wv, bv, Wo, bo):
    """Host-side prep: returns (in_maps, plan, nmt, has_vbias)."""
    Wkv = (Wk.astype(np.float64) @ Wv.astype(np.float64)).astype(np.float32)
    bkv = (bk.astype(np.float64) @ Wv.astype(np.float64)
           + bv.astype(np.float64)).astype(np.float32)
    has_vbias = bool(np.any(bkv != 0.0))

    plan, tiles_list = _classify_mask(np.asarray(mask))
    nmt = len(tiles_list)
    assert nmt <= 64, f"too many distinct mask tiles ({nmt})"
    if nmt > 0:
        mtiles = np.stack(tiles_list).astype(ml_dtypes.bfloat16)
    else:
        mtiles = np.zeros((1, 128, 128), dtype=ml_dtypes.bfloat16)

    in_maps = []
    for c in range(NCORES):
        b, g = c // 4, c % 4
        cols = slice(HF * g, HF * (g + 1))
        in_maps.append({
            "xqT": np.ascontiguousarray(queries[b].T),
            "xkT": np.ascontiguousarray(keys[b].T),
            "wq": np.ascontiguousarray(Wq[:, cols]),
            "wk": np.ascontiguousarray(Wk[:, cols]),
            "wkv": np.ascontiguousarray(Wkv[:, cols]),
            "wo": np.ascontiguousarray(Wo[cols, :]).astype(ml_dtypes.bfloat16),
            "bq": np.ascontiguousarray(bq[cols].reshape(2, 128).T),
            "bk": np.ascontiguousarray(bk[cols].reshape(2, 128).T),
            "bkv": bkv[cols].reshape(1, HF).copy(),
            "mtiles": mtiles,
        })
    return in_maps, plan, nmt, has_vbias


def combine_outputs(results, bo):
    out = np.empty((B, S, D), dtype=np.float32)
    for b in range(B):
        acc = results[4 * b]["out"].astype(np.float32).copy()
        for g in range(1, 4):
            acc += results[4 * b + g]["out"]
        out[b] = acc + bo[None, :]
    return out


def kernel(queries, keys, values, mask, Wq, bq, Wk, bk, Wv, bv, Wo, bo,
           _trace=False, _result_holder=None):
    queries = np.asarray(queries, dtype=np.float32)
    keys = np.asarray(keys, dtype=np.float32)
    mask = np.asarray(mask)
    in_maps, plan, nmt, has_vbias = shard_inputs(
        queries, keys, mask,
        np.asarray(Wq, np.float32), np.asarray(bq, np.float32),
        np.asarray(Wk, np.float32), np.asarray(bk, np.float32),
        np.asarray(Wv, np.float32), np.asarray(bv, np.float32),
        np.asarray(Wo, np.float32), np.asarray(bo, np.float32))
    nc = _get_nc(plan, nmt, has_vbias)
    res = run_bass_kernel_spmd(nc, in_maps, core_ids=list(range(NCORES)),
                               trace=_trace)
    if _result_holder is not None:
        _result_holder.append(res)
    return combine_outputs(res.results, np.asarray(bo, np.float32))
